# revision 1
# baseline (speedup 1.0000x reference)
"""Bass/Trainium2 kernel for nn_Attention_19481971654841.

Full attention block: q/k/v proj + per-head RMSNorm(q,k) + RoPE + causal GQA
SDPA + o_proj.  B=2, L=2048, D=1024, H=16, KVH=8, HD=128.

Sharding (8 NeuronCores): data-parallel over batch (2 groups of 4 cores) x
4-way tensor-parallel over heads inside each group.  Core c handles batch
c//4 and q-heads [4g:4g+4), kv-heads [2g:2g+2) with g = c%4.  Each core
produces a partial [L, D] o_proj contribution; host sums the 4 partials per
batch.

Per-core dataflow (all matmuls bf16, fp32 PSUM accumulate):
  - projections produce Q^T/K^T head-major [HD=128 part, T] (weights
    stationary on PE); V token-major [T, HD] via X-stationary matmuls.
  - RMSNorm in head-major: sum-of-squares via all-ones matmul (partition
    reduce, broadcast to 128 rows), ACT Sqrt, DVE fast reciprocal, fused
    (q * w) * r via scalar_tensor_tensor.
  - RoPE with host-prepared phase tables (args range-reduced to (-pi, pi]);
    sin table carries the rotate-half sign per partition so rope is
    4 full-width DVE ops per plane.
  - attention in S^T orientation: scores[kv, q] = K_hm^T . Q_hm, causal mask
    folded in as an identity-matmul bias add, ACT Exp -> P^T bf16, softmax
    denominator via all-ones matmul (broadcast rows), PV accumulates
    attn^T[hd, q] directly (no transposes anywhere).
  - o_proj: attn^T slices are the stationary lhsT; partial out written fp32
    from PSUM straight to DRAM.
"""

import math
from contextlib import ExitStack

import numpy as np
import ml_dtypes

import concourse.bass as bass
import concourse.mybir as mybir
import concourse.tile as tile

F32 = mybir.dt.float32
BF16 = mybir.dt.bfloat16
AF = mybir.ActivationFunctionType
ALU = mybir.AluOpType

# problem constants (per spec; hardcoded — kernel.py must be self-contained)
B, L, D = 2, 2048, 1024
H, KVH, HD = 16, 8, 128
EPS = 1e-6
ROPE_BASE = 1000000.0

# per-core constants
NCORES = 8
TPG = 4                 # tensor-parallel group size (cores per batch)
T = L                   # tokens per core (one batch element)
NQ = H // TPG           # 4 q heads per core
NKV = KVH // TPG        # 2 kv heads per core
DCH = D // 128          # 8 input-dim chunks
QT = 1024               # q tile width (PSUM tile [128, 1024] f32 = 2 banks)
NQT = T // QT           # 2 q tiles
NCH = T // 128          # 16 token chunks
MASK_NEG = -30000.0


def _chunks512(c0, end):
    """bank-aligned matmul col chunks covering [c0, end)"""
    out = []
    n0 = c0
    while n0 < end:
        ne = min(end, (n0 // 512 + 1) * 512)
        out.append((n0, ne))
        n0 = ne
    return out


def build_nc(parts="full"):
    nc = bass.Bass()

    xt_d = nc.dram_tensor("xt", [DCH, 128, T], BF16, kind="ExternalInput")
    wq_d = nc.dram_tensor("wq", [DCH, 128, NQ * HD], BF16, kind="ExternalInput")
    wk_d = nc.dram_tensor("wk", [DCH, 128, NKV * HD], BF16, kind="ExternalInput")
    wv_d = nc.dram_tensor("wv", [DCH, 128, NKV * HD], BF16, kind="ExternalInput")
    wo_d = nc.dram_tensor("wo", [NQ, 128, D], BF16, kind="ExternalInput")
    fqc_d = nc.dram_tensor("fqc", [128, T], F32, kind="ExternalInput")
    fqs_d = nc.dram_tensor("fqs", [128, T], F32, kind="ExternalInput")
    qnw_d = nc.dram_tensor("qnw", [128, 1], F32, kind="ExternalInput")
    knw_d = nc.dram_tensor("knw", [128, 1], F32, kind="ExternalInput")
    ones_d = nc.dram_tensor("ones", [128, 128], BF16, kind="ExternalInput")
    ident_d = nc.dram_tensor("ident", [128, 128], BF16, kind="ExternalInput")
    tri_d = nc.dram_tensor("tri", [128, 128], BF16, kind="ExternalInput")
    perm_d = nc.dram_tensor("perm", [128, 128], BF16, kind="ExternalInput")
    out_d = nc.dram_tensor("out", [NCH, 128, D], F32, kind="ExternalOutput")

    with tile.TileContext(nc) as tc, ExitStack() as ctx:
        sing = ctx.enter_context(tc.tile_pool(name="sing", bufs=1))
        trans = ctx.enter_context(tc.tile_pool(name="trans", bufs=2))
        pts = ctx.enter_context(tc.tile_pool(name="pts", bufs=4))
        psum = ctx.enter_context(tc.tile_pool(name="psum", bufs=1, space="PSUM"))

        # ---- persistent loads (wk/wq first, xt chunked: the first proj
        # matmuls only need wk + xt chunk 0, so PE starts ~3us in) ---------
        wq = sing.tile([128, DCH, NQ * HD], BF16, tag="wq")
        wk = sing.tile([128, DCH, NKV * HD], BF16, tag="wk")
        wv = sing.tile([128, DCH, NKV * HD], BF16, tag="wv")
        nc.sync.dma_start(out=wk, in_=wk_d.rearrange("d p f -> p d f"))
        xt = sing.tile([128, DCH, T], BF16, tag="xt")
        for d in range(DCH):
            nc.sync.dma_start(out=xt[:, d, :], in_=xt_d[d])
        nc.sync.dma_start(out=wq, in_=wq_d.rearrange("d p f -> p d f"))
        nc.sync.dma_start(out=wv, in_=wv_d.rearrange("d p f -> p d f"))
        wo = sing.tile([128, NQ, D], BF16, tag="wo")
        nc.sync.dma_start(out=wo, in_=wo_d.rearrange("h p f -> p h f"))
        fqc = sing.tile([128, T], F32, tag="fqc")
        fqs = sing.tile([128, T], F32, tag="fqs")
        nc.sync.dma_start(out=fqc, in_=fqc_d[:, :])
        nc.sync.dma_start(out=fqs, in_=fqs_d[:, :])
        qnw = sing.tile([128, 1], F32, tag="qnw")
        knw = sing.tile([128, 1], F32, tag="knw")
        nc.sync.dma_start(out=qnw, in_=qnw_d[:, :])
        nc.sync.dma_start(out=knw, in_=knw_d[:, :])
        ones = sing.tile([128, 128], BF16, tag="ones")
        ident = sing.tile([128, 128], BF16, tag="ident")
        tri = sing.tile([128, 128], BF16, tag="tri")
        perm = sing.tile([128, 128], BF16, tag="perm")
        nc.sync.dma_start(out=ones, in_=ones_d[:, :])
        nc.sync.dma_start(out=ident, in_=ident_d[:, :])
        nc.sync.dma_start(out=tri, in_=tri_d[:, :])
        nc.sync.dma_start(out=perm, in_=perm_d[:, :])
        epsb = sing.tile([128, 1], F32, tag="epsb")
        nc.vector.memset(epsb, EPS)

        # ---- rope tables: C = cos(pos*invf), Ssig = +-sin (sign folded) --
        ctab = sing.tile([128, T], BF16, tag="ctab")
        stab = sing.tile([128, T], BF16, tag="stab")
        nc.scalar.activation(out=ctab, in_=fqc, func=AF.Sin)
        nc.scalar.activation(out=stab, in_=fqs, func=AF.Sin)

        # ---- persistent plane outputs -----------------------------------
        khm = [sing.tile([128, T], BF16, tag=f"khm{i}", name=f"khm{i}") for i in range(NKV)]
        qhm = [sing.tile([128, T], BF16, tag=f"qhm{i}", name=f"qhm{i}") for i in range(NQ)]
        vsb = sing.tile([128, NKV, T], BF16, tag="vsb")
        attn = [sing.tile([128, T], BF16, tag=f"attn{i}", name=f"attn{i}") for i in range(NQ)]

        # ---- norm + rope pipeline, split into stages so plane units can
        # be software-pipelined against each other and against attention.
        # Unit = (plane, half).  Stage F (front): projection matmuls + early
        # PSUM drain to bf16 + square.  Stage B (back): partition-reduce
        # sum-of-squares (ones matmul), sqrt, reciprocal, fused norm scale,
        # rotate-half matmul, rope multiplies.
        def unit_front(w_ap, wmat, fslice, half):
            qp = psum.tile([128, QT], F32, tag="pp", bufs=2, name="qp")
            for d in range(DCH):
                for n0 in range(0, QT, 512):
                    nc.tensor.matmul(
                        qp[:, n0:n0 + 512],
                        lhsT=wmat[:, d, fslice],
                        rhs=xt[:, d, half * QT + n0: half * QT + n0 + 512],
                        start=(d == 0), stop=(d == DCH - 1),
                    )
            qc = trans.tile([128, QT], BF16, tag="qc", bufs=3, name="qc")
            nc.vector.tensor_copy(qc, qp)      # early PSUM drain
            sq = trans.tile([128, QT], BF16, tag="sq", bufs=2, name="sq")
            nc.vector.tensor_mul(sq, qc, qc)
            return (w_ap, half, qc, sq)

        def unit_back(plane_out, st):
            w_ap, half, qc, sq = st
            ssq = psum.tile([128, QT], F32, tag="pacc", bufs=1, name="ssq")
            for n0 in range(0, QT, 512):
                nc.tensor.matmul(ssq[:, n0:n0 + 512], lhsT=ones,
                                 rhs=sq[:, n0:n0 + 512], start=True, stop=True)
            ss = trans.tile([128, QT], F32, tag="ss", name="ss")
            nc.scalar.activation(out=ss, in_=ssq, func=AF.Sqrt,
                                 scale=1.0 / HD, bias=epsb)
            rr = trans.tile([128, QT], F32, tag="rr", name="rr")
            nc.vector.reciprocal(out=rr, in_=ss)
            qn = trans.tile([128, QT], BF16, tag="qn", name="qn")
            nc.vector.scalar_tensor_tensor(
                out=qn, in0=qc, scalar=w_ap, in1=rr,
                op0=ALU.mult, op1=ALU.mult)
            # rope: rotate-half via permutation matmul on PE, sign is
            # folded into the sin table (stab rows 0:64 hold -sin)
            cs = slice(half * QT, half * QT + QT)
            mc = trans.tile([128, QT], BF16, tag="mc", name="mc")
            nc.vector.tensor_mul(mc, qn, ctab[:, cs])
            rot = psum.tile([128, QT], F32, tag="pd", bufs=1, name="rot")
            for n0 in range(0, QT, 512):
                nc.tensor.matmul(rot[:, n0:n0 + 512], lhsT=perm,
                                 rhs=qn[:, n0:n0 + 512],
                                 start=True, stop=True)
            msw = trans.tile([128, QT], BF16, tag="msw", name="msw")
            nc.vector.tensor_mul(msw, rot, stab[:, cs])
            # final rope add on the (otherwise idle) gpsimd engine
            nc.gpsimd.tensor_add(plane_out[:, cs], mc, msw)

        # kv plane units, depth-2 pipelined
        kunits = [(khm[i], knw, wk, slice(i * HD, (i + 1) * HD), half)
                  for i in range(NKV) for half in range(NQT)]
        kunits += [(qhm[0], qnw, wq, slice(0, HD), half)
                   for half in range(NQT)]
        pend = []
        for plane, w_ap, wmat, fsl, half in kunits:
            st = unit_front(w_ap, wmat, fsl, half)
            pend.append((plane, st))
            if len(pend) > 1:
                unit_back(*pend.pop(0))
        while pend:
            unit_back(*pend.pop(0))

        def v_chunks(cs):
            # V projection: token-major via X-stationary matmuls, both kv
            for c in cs:
                vp = psum.tile([128, NKV * HD], F32, tag="pacc", bufs=1,
                               name="vp")
                for d in range(DCH):
                    nc.tensor.matmul(
                        vp, lhsT=xt[:, d, c * 128:(c + 1) * 128],
                        rhs=wv[:, d, :], start=(d == 0), stop=(d == DCH - 1))
                nc.vector.tensor_copy(
                    vsb[:, :, c * 128:(c + 1) * 128],
                    vp.rearrange("p (k t) -> p k t", k=NKV))

        def q_unit_front(i, half):
            return unit_front(qnw, wq, slice(i * HD, (i + 1) * HD), half)

        # ---- attention (S^T orientation), software-pipelined j loop ------
        # emit denom/PV of block j-1 after scores of block j so the PE queue
        # never stalls on the current block's exp (ACT).
        def attn_qt(h, iqt):
            kv = h // 2
            ps_o = psum.tile([128, QT], F32, tag="pacc", bufs=1, name="ps_o")
            ps_d = psum.tile([128, QT], F32, tag="pd", bufs=1, name="ps_d")
            jmax = 8 * iqt + 8

            def scores(j):
                c0 = max(0, 128 * j - QT * iqt)
                c0a = (c0 // 512) * 512   # bank-aligned start for denom/PV
                ks = slice(128 * j, 128 * j + 128)
                ps_s = psum.tile([128, QT], F32, tag="pp", bufs=2, name="ps_s")
                diag = j >= 8 * iqt
                # compute scores + exp only from the exact causal start c0
                # (128-aligned, still within one bank per chunk)
                for n0, ne in _chunks512(c0, QT):
                    first = diag and n0 == c0
                    nc.tensor.matmul(
                        ps_s[:, n0:ne], lhsT=khm[kv][:, ks],
                        rhs=qhm[h][:, iqt * QT + n0: iqt * QT + ne],
                        start=True, stop=not first)
                    if first:
                        # triangular -inf on the diagonal block [c0, c0+128)
                        nc.tensor.matmul(
                            ps_s[:, c0:c0 + 128], lhsT=ident,
                            rhs=tri, start=False, stop=True)
                pt = pts.tile([128, QT], BF16, tag="pt", name="pt")
                if c0 > c0a:
                    # denom/PV read bank-aligned [c0a:); zero the pad region
                    nc.gpsimd.memset(pt[:, c0a:c0], 0.0)
                nc.scalar.activation(out=pt[:, c0:QT], in_=ps_s[:, c0:QT],
                                     func=AF.Exp)
                return j, c0a, pt

            def denoms(st):
                j, c0a, pt = st
                for n0, ne in _chunks512(c0a, QT):
                    jl = min(jmax - 1, 8 * iqt + n0 // 128 + 3)
                    nc.tensor.matmul(ps_d[:, n0:ne], lhsT=ones,
                                     rhs=pt[:, n0:ne],
                                     start=(j == 0), stop=(j == jl))

            def pvs(st):
                j, c0a, pt = st
                kvs = slice(128 * j, 128 * j + 128)
                for n0, ne in _chunks512(c0a, QT):
                    jl = min(jmax - 1, 8 * iqt + n0 // 128 + 3)
                    nc.tensor.matmul(ps_o[:, n0:ne],
                                     lhsT=vsb[:, kv, kvs],
                                     rhs=pt[:, n0:ne],
                                     start=(j == 0), stop=(j == jl))

            # pair-pipelined: scores for blocks (j, j+1), then the previous
            # pair's denominators back-to-back (ones lhsT dedupes) and PVs
            prev = None
            for jp in range(0, jmax, 2):
                st0 = scores(jp)
                st1 = scores(jp + 1)
                if prev is not None:
                    denoms(prev[0]); denoms(prev[1])
                    pvs(prev[0]); pvs(prev[1])
                prev = (st0, st1)
            denoms(prev[0]); denoms(prev[1])
            pvs(prev[0]); pvs(prev[1])
            rb = trans.tile([128, QT], F32, tag="rb", name="rb")
            nc.vector.reciprocal(out=rb, in_=ps_d)
            nc.vector.tensor_mul(
                attn[h][:, iqt * QT:(iqt + 1) * QT], ps_o, rb)

        def o_proj(c):
            po = psum.tile([128, D], F32, tag=("pp" if c % 2 == 0 else "pd"),
                           bufs=2 if c % 2 == 0 else 1, name="po")
            ts = slice(c * 128, (c + 1) * 128)
            for hh in range(NQ):
                for n0 in range(0, D, 512):
                    nc.tensor.matmul(po[:, n0:n0 + 512],
                                     lhsT=attn[hh][:, ts],
                                     rhs=wo[:, hh, n0:n0 + 512],
                                     start=(hh == 0), stop=(hh == NQ - 1))
            ob = pts.tile([128, D], F32, tag="ob", bufs=2, name="ob")
            nc.vector.tensor_copy(ob, po)
            nc.sync.dma_start(out=out_d[c], in_=ob)

        # Interleave: while attention for plane h runs (PE+ACT heavy), emit
        # the next q plane's projection front (PE) and, after, its norm/rope
        # back stage (DVE heavy).  o_proj for the first half overlaps the
        # second half's attention; its output DMA overlaps everything.
        if parts != "planes":
            v_chunks(range(NCH))
            for h in range(NQ):
                nxt = []
                if h + 1 < NQ:
                    nxt = [q_unit_front(h + 1, 0), q_unit_front(h + 1, 1)]
                attn_qt(h, 0)
                if h == NQ - 1 and parts != "noproj":
                    for c in range(8):
                        o_proj(c)
                if nxt:
                    unit_back(qhm[h + 1], nxt[0])
                attn_qt(h, 1)
                if nxt:
                    unit_back(qhm[h + 1], nxt[1])
            if parts != "noproj":
                for c in range(8, NCH):
                    o_proj(c)

    return nc


def legalize_waits(bir_bytes):
    """This walrus build rejects compute instructions with more than one
    sync wait.  Hoist all but one wait of each instruction into standalone
    EventSemaphore (pure wait) instructions on the same engine queue, which
    is semantically identical (in-order engine queues)."""
    import json
    m = json.loads(bir_bytes)
    n_fix = 0
    for f in m["functions"]:
        for blk in f["blocks"]:
            # drop Ldweights identical to the previously-kept one (the
            # stationary operand is still loaded; bass re-emits per matmul).
            # Safe: Ldweights carry no on_update; waits (rare) are kept.
            out0 = []
            last_key = None
            for ins in blk["instructions"]:
                if ins["opcode"] == "Ldweights":
                    si = ins.get("sync_info") or {}
                    key = json.dumps(
                        [ins.get("ins"), ins.get("outs"),
                         ins.get("perf_mode"), ins.get("tile_position")])
                    if (key == last_key and not si.get("on_wait")
                            and not si.get("on_update")):
                        continue
                    last_key = key
                out0.append(ins)
            blk["instructions"] = out0
            out = []
            for ins in blk["instructions"]:
                si = ins.get("sync_info")
                waits = (si or {}).get("on_wait") or []
                if len(waits) > 1 and ins["opcode"] != "EventSemaphore":
                    for i, w in enumerate(waits[:-1]):
                        out.append({
                            "debug": ins.get("debug", 0),
                            "engine": ins["engine"],
                            "ins": [], "outs": [],
                            "name": f"{ins['name']}-hw{i}",
                            "opcode": "EventSemaphore",
                            "sync_info": {"on_update": [], "on_wait": [w]},
                        })
                    si["on_wait"] = [waits[-1]]
                    n_fix += 1
                out.append(ins)
            blk["instructions"] = out
    return json.dumps(m).encode()


def _wrap_pi(x):
    return np.mod(x + np.pi, 2 * np.pi) - np.pi


def _prep_core_inputs(c, hidden_states, position_ids, q_w, k_w, v_w, o_w,
                      q_norm_w, k_norm_w):
    b, g = divmod(c, TPG)
    bf = ml_dtypes.bfloat16
    xt = np.ascontiguousarray(
        np.asarray(hidden_states[b], np.float32).T).astype(bf).reshape(DCH, 128, T)
    wq = np.ascontiguousarray(
        np.asarray(q_w[NQ * HD * g: NQ * HD * (g + 1)], np.float32).T
    ).astype(bf).reshape(DCH, 128, NQ * HD)
    wk = np.ascontiguousarray(
        np.asarray(k_w[NKV * HD * g: NKV * HD * (g + 1)], np.float32).T
    ).astype(bf).reshape(DCH, 128, NKV * HD)
    wv = np.ascontiguousarray(
        np.asarray(v_w[NKV * HD * g: NKV * HD * (g + 1)], np.float32).T
    ).astype(bf).reshape(DCH, 128, NKV * HD)
    wo = np.ascontiguousarray(
        np.asarray(o_w[:, NQ * HD * g: NQ * HD * (g + 1)], np.float32).T
    ).astype(bf).reshape(NQ, 128, D)

    pos = np.asarray(position_ids[b], np.float64)                      # [T]
    inv = 1.0 / (ROPE_BASE ** (np.arange(0, HD, 2, dtype=np.float64) / HD))
    invf2 = np.concatenate([inv, inv])                                 # [128]
    invf2s = np.concatenate([-inv, inv])
    ph = pos[None, :] * invf2[:, None]
    phs = pos[None, :] * invf2s[:, None]
    fqc = _wrap_pi(ph + np.pi / 2).astype(np.float32)      # sin(x)=cos(phase)
    fqs = _wrap_pi(phs).astype(np.float32)                 # signed sin
    qnw = (np.asarray(q_norm_w, np.float32) / math.sqrt(HD)).reshape(128, 1)
    knw = np.asarray(k_norm_w, np.float32).reshape(128, 1)

    ones = np.ones((128, 128), bf)
    ident = np.eye(128, dtype=np.float32).astype(bf)
    tri = np.where(np.arange(128)[:, None] <= np.arange(128)[None, :],
                   0.0, MASK_NEG).astype(np.float32).astype(bf)
    perm = np.zeros((128, 128), np.float32)
    perm[(np.arange(128) + 64) % 128, np.arange(128)] = 1.0
    perm = perm.astype(bf)
    return dict(xt=xt, wq=wq, wk=wk, wv=wv, wo=wo, fqc=fqc, fqs=fqs,
                qnw=qnw, knw=knw, ones=ones, ident=ident, tri=tri,
                perm=perm)


def kernel(hidden_states, position_ids, q_w, k_w, v_w, o_w, q_norm_w,
           k_norm_w):
    from concourse.bass_utils import run_bass_kernel_spmd

    nc = build_nc()
    orig_ser = nc.to_json_bytes
    nc.to_json_bytes = lambda: legalize_waits(orig_ser())
    in_maps = [
        _prep_core_inputs(c, hidden_states, position_ids, q_w, k_w, v_w, o_w,
                          q_norm_w, k_norm_w)
        for c in range(NCORES)
    ]
    res = run_bass_kernel_spmd(nc, in_maps, list(range(NCORES))).results
    out = np.zeros((B, L, D), np.float32)
    for c in range(NCORES):
        out[c // TPG] += np.asarray(res[c]["out"], np.float32).reshape(L, D)
    return out



# revision 10
# speedup vs baseline: 212.7712x; 212.7712x over previous
"""Bass/Trainium2 kernel for nn_Attention_19481971654841.

Full attention block: q/k/v proj + per-head RMSNorm(q,k) + RoPE + causal GQA
SDPA + o_proj.  B=2, L=2048, D=1024, H=16, KVH=8, HD=128.

Sharding (8 NeuronCores): data-parallel over batch (2 groups of 4 cores) x
4-way tensor-parallel over heads inside each group.  Core c handles batch
c//4 and q-heads [4g:4g+4), kv-heads [2g:2g+2) with g = c%4.  Each core
produces a partial [L, D] o_proj contribution; host sums the 4 partials per
batch.

Per-core dataflow (all matmuls bf16, fp32 PSUM accumulate), in three strict
phases chosen so the ACT engine never thrashes activation tables (Sqrt and
Exp live in different tables; a switch costs 1.3us):

Phase 1 - projections + norm + rope (PE-bound, ~60us):
  - Q^T/K^T head-major [HD=128 part, T] (weights stationary); V token-major
    via X-stationary matmuls.
  - RMSNorm: sum-of-squares via all-ones matmul (partition reduce broadcast
    to 128 rows), ACT Sqrt (batched - one table load), DVE reciprocal,
    fused (q * w) * r scalar_tensor_tensor.
  - RoPE: host-prepared bf16 cos/sin tables (sin carries the rotate-half
    sign); rotate-half itself is a partition swap done by SBUF->SBUF DMA,
    so rope is 2 fast DVE muls + a Pool add per plane unit.

Phase 2 - attention (PE/ACT-balanced, ~100us), S^T orientation:
  scores[kv, q] = K_hm^T . Q_hm -> ACT Exp (one table load, accum-free) ->
  P^T bf16; causal diagonal masked by a Pool-engine multiply with an
  upper-triangular 0/1 block AFTER exp (no PE mask matmuls); softmax
  denominator via all-ones matmul; PV accumulates attn^T[hd, q] directly.
  All 96 (head, qtile, kvblock) units flow through one software pipeline
  (scores of pair i+1 emitted before denom/PV of pair i) with no per-head
  drains.

Phase 3 - o_proj tail (~28us): attn^T slices stationary, fp32 partial out
  DMA'd straight from PSUM to DRAM (no drain copies), double-buffered.
"""

import math
from contextlib import ExitStack

import numpy as np
import ml_dtypes

import concourse.bass as bass
import concourse.mybir as mybir
import concourse.tile as tile

F32 = mybir.dt.float32
BF16 = mybir.dt.bfloat16
AF = mybir.ActivationFunctionType
ALU = mybir.AluOpType

# problem constants (per spec; hardcoded — kernel.py must be self-contained)
B, L, D = 2, 2048, 1024
H, KVH, HD = 16, 8, 128
EPS = 1e-6
ROPE_BASE = 1000000.0

# per-core constants
NCORES = 8
TPG = 4                 # tensor-parallel group size (cores per batch)
T = L                   # tokens per core (one batch element)
NQ = H // TPG           # 4 q heads per core
NKV = KVH // TPG        # 2 kv heads per core
DCH = D // 128          # 8 input-dim chunks
QT = 1024               # q tile width (PSUM tile [128, 1024] f32 = 2 banks)
NQT = T // QT           # 2 q tiles
NCH = T // 128          # 16 token chunks


def _chunks512(c0, end):
    """bank-aligned matmul col chunks covering [c0, end)"""
    out = []
    n0 = c0
    while n0 < end:
        ne = min(end, (n0 // 512 + 1) * 512)
        out.append((n0, ne))
        n0 = ne
    return out


def build_nc(nrep=1):
    """nrep > 1 replicates the whole kernel body (same SBUF buffers, fresh
    DRAM loads) for benchmarking: per-rep steady-state time = true HW exec
    time with dispatch overhead amortized away.  kernel() always uses
    nrep=1."""
    nc = bass.Bass()

    xt_d = nc.dram_tensor("xt", [DCH, 128, T], BF16, kind="ExternalInput")
    wq_d = nc.dram_tensor("wq", [DCH, 128, NQ * HD], BF16, kind="ExternalInput")
    wk_d = nc.dram_tensor("wk", [DCH, 128, NKV * HD], BF16, kind="ExternalInput")
    wv_d = nc.dram_tensor("wv", [DCH, 128, NKV * HD], BF16, kind="ExternalInput")
    wo_d = nc.dram_tensor("wo", [NQ, 128, D], BF16, kind="ExternalInput")
    ctab_d = nc.dram_tensor("ctab", [128, T], BF16, kind="ExternalInput")
    stab_d = nc.dram_tensor("stab", [128, T], BF16, kind="ExternalInput")
    qnw_d = nc.dram_tensor("qnw", [128, 1], F32, kind="ExternalInput")
    knw_d = nc.dram_tensor("knw", [128, 1], F32, kind="ExternalInput")
    ones_d = nc.dram_tensor("ones", [128, 128], BF16, kind="ExternalInput")
    tri01_d = nc.dram_tensor("tri01", [128, 128], BF16, kind="ExternalInput")
    out_d = nc.dram_tensor("out", [NCH, 128, D], F32, kind="ExternalOutput")

    with tile.TileContext(nc) as tc, ExitStack() as ctx:
        sing = ctx.enter_context(tc.tile_pool(name="sing", bufs=1))
        trans = ctx.enter_context(tc.tile_pool(name="trans", bufs=2))
        pts = ctx.enter_context(tc.tile_pool(name="pts", bufs=4))
        psum = ctx.enter_context(tc.tile_pool(name="psum", bufs=1, space="PSUM"))

        for _rep in range(nrep):
            # ---- persistent loads (wk + xt chunk 0 first so PE starts early)
            wk = sing.tile([128, DCH, NKV * HD], BF16, tag="wk")
            nc.sync.dma_start(out=wk, in_=wk_d.rearrange("d p f -> p d f"))
            # xt split across the two HWDGE queues (SP + ACT) so the first
            # projection fronts aren't DMA-bandwidth starved
            xt = sing.tile([128, DCH, T], BF16, tag="xt")
            for d in range(DCH):
                eng = nc.sync if d % 2 == 0 else nc.scalar
                eng.dma_start(out=xt[:, d, :], in_=xt_d[d])
            wq = sing.tile([128, DCH, NQ * HD], BF16, tag="wq")
            wv = sing.tile([128, DCH, NKV * HD], BF16, tag="wv")
            nc.scalar.dma_start(out=wq, in_=wq_d.rearrange("d p f -> p d f"))
            nc.sync.dma_start(out=wv, in_=wv_d.rearrange("d p f -> p d f"))
            wo = sing.tile([128, NQ, D], BF16, tag="wo")
            nc.sync.dma_start(out=wo, in_=wo_d.rearrange("h p f -> p h f"))
            ctab = sing.tile([128, T], BF16, tag="ctab")
            stab = sing.tile([128, T], BF16, tag="stab")
            nc.sync.dma_start(out=ctab, in_=ctab_d[:, :])
            nc.sync.dma_start(out=stab, in_=stab_d[:, :])
            qnw = sing.tile([128, 1], F32, tag="qnw")
            knw = sing.tile([128, 1], F32, tag="knw")
            nc.sync.dma_start(out=qnw, in_=qnw_d[:, :])
            nc.sync.dma_start(out=knw, in_=knw_d[:, :])
            ones = sing.tile([128, 128], BF16, tag="ones")
            tri01 = sing.tile([128, 128], BF16, tag="tri01")
            nc.sync.dma_start(out=ones, in_=ones_d[:, :])
            nc.sync.dma_start(out=tri01, in_=tri01_d[:, :])
            epsb = sing.tile([128, 1], F32, tag="epsb")
            nc.vector.memset(epsb, EPS)

            # ---- persistent plane outputs -------------------------------
            khm = [sing.tile([128, T], BF16, tag=f"khm{i}", name=f"khm{i}")
                   for i in range(NKV)]
            qhm = [sing.tile([128, T], BF16, tag=f"qhm{i}", name=f"qhm{i}")
                   for i in range(NQ)]
            vsb = sing.tile([128, NKV, T], BF16, tag="vsb")
            attn = [sing.tile([128, T], BF16, tag=f"attn{i}", name=f"attn{i}")
                    for i in range(NQ)]

            # ---- phase 1: projections + norm + rope ---------------------
            # Unit = (plane, half).  Front: projection matmuls + PSUM drain
            # to bf16 + square.  Back: sum-of-squares ones-matmul, ACT Sqrt,
            # DVE reciprocal + fused scale, rope (DMA partition swap + DVE
            # muls + Pool add).
            def unit_front(w_ap, wmat, fslice, half):
                qp = psum.tile([128, QT], F32, tag="pp", bufs=2, name="qp")
                for d in range(DCH):
                    for n0 in range(0, QT, 512):
                        nc.tensor.matmul(
                            qp[:, n0:n0 + 512],
                            lhsT=wmat[:, d, fslice],
                            rhs=xt[:, d, half * QT + n0: half * QT + n0 + 512],
                            start=(d == 0), stop=(d == DCH - 1),
                        )
                # drain qp on ACT (Square + Copy live in the Sqrt table, so
                # phase 1 stays on one ACT table); DVE keeps the norm chain
                sq = trans.tile([128, QT], BF16, tag="sq", bufs=2, name="sq")
                nc.scalar.activation(out=sq, in_=qp, func=AF.Square)
                qc = trans.tile([128, QT], BF16, tag="qc", bufs=3, name="qc")
                nc.scalar.activation(out=qc, in_=qp, func=AF.Copy)
                return (w_ap, half, qc, sq)

            def unit_back(plane_out, st):
                w_ap, half, qc, sq = st
                ssq = psum.tile([128, QT], F32, tag="pd", bufs=1, name="ssq")
                for n0 in range(0, QT, 512):
                    nc.tensor.matmul(ssq[:, n0:n0 + 512], lhsT=ones,
                                     rhs=sq[:, n0:n0 + 512],
                                     start=True, stop=True)
                ss = trans.tile([128, QT], F32, tag="ss", name="ss")
                nc.scalar.activation(out=ss, in_=ssq, func=AF.Sqrt,
                                     scale=1.0 / HD, bias=epsb)
                rr = trans.tile([128, QT], F32, tag="rr", name="rr")
                nc.vector.reciprocal(out=rr, in_=ss)
                qn = trans.tile([128, QT], BF16, tag="qn", name="qn")
                nc.vector.scalar_tensor_tensor(
                    out=qn, in0=qc, scalar=w_ap, in1=rr,
                    op0=ALU.mult, op1=ALU.mult)
                cs = slice(half * QT, half * QT + QT)
                mc = trans.tile([128, QT], BF16, tag="mc", name="mc")
                nc.vector.tensor_mul(mc, qn, ctab[:, cs])
                # rotate-half: partition swap via SBUF->SBUF DMA; the sign
                # lives in the sin table (stab rows 0:64 hold -sin)
                qnsw = trans.tile([128, QT], BF16, tag="qnsw", name="qnsw")
                nc.sync.dma_start(out=qnsw[0:64, :], in_=qn[64:128, :])
                nc.sync.dma_start(out=qnsw[64:128, :], in_=qn[0:64, :])
                msw = trans.tile([128, QT], BF16, tag="msw", name="msw")
                nc.vector.tensor_mul(msw, qnsw, stab[:, cs])
                nc.gpsimd.tensor_add(plane_out[:, cs], mc, msw)

            units = [(khm[i], knw, wk, slice(i * HD, (i + 1) * HD), half)
                     for i in range(NKV) for half in range(NQT)]
            units += [(qhm[i], qnw, wq, slice(i * HD, (i + 1) * HD), half)
                      for i in range(NQ) for half in range(NQT)]
            pend = []
            for plane, w_ap, wmat, fsl, half in units:
                st = unit_front(w_ap, wmat, fsl, half)
                pend.append((plane, st))
                if len(pend) > 1:
                    unit_back(*pend.pop(0))
            while pend:
                unit_back(*pend.pop(0))

            # V projection: token-major via X-stationary matmuls
            for c in range(NCH):
                vp = psum.tile([128, NKV * HD], F32, tag="pacc", bufs=1,
                               name="vp")
                for d in range(DCH):
                    nc.tensor.matmul(
                        vp, lhsT=xt[:, d, c * 128:(c + 1) * 128],
                        rhs=wv[:, d, :], start=(d == 0), stop=(d == DCH - 1))
                nc.scalar.activation(
                    out=vsb[:, :, c * 128:(c + 1) * 128],
                    in_=vp.rearrange("p (k t) -> p k t", k=NKV),
                    func=AF.Copy)

            # ---- phase 2: attention, one global software pipeline -------
            # Block = (h, iqt, j).  Pairs (j, j+1) within a (h, iqt) unit;
            # scores+exp of pair i+1 are emitted before denom/PV of pair i
            # so PE never waits on ACT.  ps_o/ps_d live per (h, iqt).
            st_ctx = {}

            def scores(h, iqt, j):
                kv = h // 2
                c0 = max(0, 128 * j - QT * iqt)
                c0a = (c0 // 512) * 512   # bank-aligned start for denom/PV
                ks = slice(128 * j, 128 * j + 128)
                ps_s = psum.tile([128, QT], F32, tag="pp", bufs=2, name="ps_s")
                for n0, ne in _chunks512(c0, QT):
                    nc.tensor.matmul(
                        ps_s[:, n0:ne], lhsT=khm[kv][:, ks],
                        rhs=qhm[h][:, iqt * QT + n0: iqt * QT + ne],
                        start=True, stop=True)
                pt = pts.tile([128, QT], BF16, tag="pt", name="pt")
                if c0 > c0a:
                    # denom/PV read bank-aligned [c0a:); zero the pad region
                    nc.gpsimd.memset(pt[:, c0a:c0], 0.0)
                nc.scalar.activation(out=pt[:, c0:QT], in_=ps_s[:, c0:QT],
                                     func=AF.Exp)
                if j >= 8 * iqt:
                    # causal diagonal: zero the strictly-lower block entries
                    nc.gpsimd.tensor_mul(pt[:, c0:c0 + 128],
                                         pt[:, c0:c0 + 128], tri01)
                return j, c0a, pt

            def denoms(h, iqt, st):
                j, c0a, pt = st
                jmax = 8 * iqt + 8
                if (h, iqt, "d") not in st_ctx:
                    # lazy alloc: first write lands after the previous
                    # unit's reciprocal (same pd buffer) was emitted
                    st_ctx[(h, iqt, "d")] = psum.tile(
                        [128, QT], F32, tag="pd", bufs=1, name="ps_d")
                ps_d = st_ctx[(h, iqt, "d")]
                for n0, ne in _chunks512(c0a, QT):
                    jl = min(jmax - 1, 8 * iqt + n0 // 128 + 3)
                    nc.tensor.matmul(ps_d[:, n0:ne], lhsT=ones,
                                     rhs=pt[:, n0:ne],
                                     start=(j == 0), stop=(j == jl))

            def pvs(h, iqt, st):
                j, c0a, pt = st
                jmax = 8 * iqt + 8
                kv = h // 2
                if (h, iqt, "o") not in st_ctx:
                    st_ctx[(h, iqt, "o")] = psum.tile(
                        [128, QT], F32, tag="pacc", bufs=1, name="ps_o")
                ps_o = st_ctx[(h, iqt, "o")]
                kvs = slice(128 * j, 128 * j + 128)
                for n0, ne in _chunks512(c0a, QT):
                    jl = min(jmax - 1, 8 * iqt + n0 // 128 + 3)
                    nc.tensor.matmul(ps_o[:, n0:ne],
                                     lhsT=vsb[:, kv, kvs],
                                     rhs=pt[:, n0:ne],
                                     start=(j == 0), stop=(j == jl))

            def finish_unit(h, iqt):
                ps_d = st_ctx.pop((h, iqt, "d"))
                ps_o = st_ctx.pop((h, iqt, "o"))
                rb = trans.tile([128, QT], F32, tag="rb", name="rb")
                nc.vector.reciprocal(out=rb, in_=ps_d)
                nc.vector.tensor_mul(
                    attn[h][:, iqt * QT:(iqt + 1) * QT], ps_o, rb)

            pairs = []
            for h in range(NQ):
                for iqt in range(NQT):
                    js = list(range(8 * iqt + 8))
                    pairs += [(h, iqt, js[i], js[i + 1])
                              for i in range(0, len(js), 2)]

            prev = None
            for h, iqt, j0, j1 in pairs:
                s0 = scores(h, iqt, j0)
                s1 = scores(h, iqt, j1)
                if prev is not None:
                    ph, piqt, p0, p1 = prev
                    denoms(ph, piqt, p0)
                    denoms(ph, piqt, p1)
                    pvs(ph, piqt, p0)
                    pvs(ph, piqt, p1)
                    if p1[0] == 8 * piqt + 8 - 1:  # last pair of unit
                        finish_unit(ph, piqt)
                prev = (h, iqt, s0, s1)
            ph, piqt, p0, p1 = prev
            denoms(ph, piqt, p0)
            denoms(ph, piqt, p1)
            pvs(ph, piqt, p0)
            pvs(ph, piqt, p1)
            finish_unit(ph, piqt)

            # ---- phase 3: o_proj; ACT (idle here, Copy is in the exp
            # table) drains PSUM, DMA ships to DRAM, double-buffered ------
            for c in range(NCH):
                po = psum.tile([128, D], F32, tag="pp", bufs=2, name="po")
                ts = slice(c * 128, (c + 1) * 128)
                for hh in range(NQ):
                    for n0 in range(0, D, 512):
                        nc.tensor.matmul(po[:, n0:n0 + 512],
                                         lhsT=attn[hh][:, ts],
                                         rhs=wo[:, hh, n0:n0 + 512],
                                         start=(hh == 0), stop=(hh == NQ - 1))
                ob = pts.tile([128, D], F32, tag="ob", bufs=2, name="ob")
                nc.scalar.activation(out=ob, in_=po, func=AF.Copy)
                nc.sync.dma_start(out=out_d[c], in_=ob)

    return nc


def legalize_waits(bir_bytes):
    """This walrus build rejects compute instructions with more than one
    sync wait.  Hoist all but one wait of each instruction into standalone
    EventSemaphore (pure wait) instructions on the same engine queue, which
    is semantically identical (in-order engine queues)."""
    import json
    m = json.loads(bir_bytes)
    n_fix = 0
    for f in m["functions"]:
        for blk in f["blocks"]:
            # drop Ldweights identical to the previously-kept one (the
            # stationary operand is still loaded; bass re-emits per matmul).
            # Safe: Ldweights carry no on_update; waits (rare) are kept.
            out0 = []
            last_key = None
            for ins in blk["instructions"]:
                if ins["opcode"] == "Ldweights":
                    si = ins.get("sync_info") or {}
                    key = json.dumps(
                        [ins.get("ins"), ins.get("outs"),
                         ins.get("perf_mode"), ins.get("tile_position")])
                    if (key == last_key and not si.get("on_wait")
                            and not si.get("on_update")):
                        continue
                    last_key = key
                out0.append(ins)
            blk["instructions"] = out0
            out = []
            for ins in blk["instructions"]:
                si = ins.get("sync_info")
                waits = (si or {}).get("on_wait") or []
                if len(waits) > 1 and ins["opcode"] != "EventSemaphore":
                    for i, w in enumerate(waits[:-1]):
                        out.append({
                            "debug": ins.get("debug", 0),
                            "engine": ins["engine"],
                            "ins": [], "outs": [],
                            "name": f"{ins['name']}-hw{i}",
                            "opcode": "EventSemaphore",
                            "sync_info": {"on_update": [], "on_wait": [w]},
                        })
                    si["on_wait"] = [waits[-1]]
                    n_fix += 1
                out.append(ins)
            blk["instructions"] = out
    return json.dumps(m).encode()


def _prep_core_inputs(c, hidden_states, position_ids, q_w, k_w, v_w, o_w,
                      q_norm_w, k_norm_w):
    b, g = divmod(c, TPG)
    bf = ml_dtypes.bfloat16
    xt = np.ascontiguousarray(
        np.asarray(hidden_states[b], np.float32).T).astype(bf).reshape(DCH, 128, T)
    wq = np.ascontiguousarray(
        np.asarray(q_w[NQ * HD * g: NQ * HD * (g + 1)], np.float32).T
    ).astype(bf).reshape(DCH, 128, NQ * HD)
    wk = np.ascontiguousarray(
        np.asarray(k_w[NKV * HD * g: NKV * HD * (g + 1)], np.float32).T
    ).astype(bf).reshape(DCH, 128, NKV * HD)
    wv = np.ascontiguousarray(
        np.asarray(v_w[NKV * HD * g: NKV * HD * (g + 1)], np.float32).T
    ).astype(bf).reshape(DCH, 128, NKV * HD)
    wo = np.ascontiguousarray(
        np.asarray(o_w[:, NQ * HD * g: NQ * HD * (g + 1)], np.float32).T
    ).astype(bf).reshape(NQ, 128, D)

    pos = np.asarray(position_ids[b], np.float64)                      # [T]
    inv = 1.0 / (ROPE_BASE ** (np.arange(0, HD, 2, dtype=np.float64) / HD))
    invf2 = np.concatenate([inv, inv])                                 # [128]
    invf2s = np.concatenate([-inv, inv])
    ctab = np.cos(pos[None, :] * invf2[:, None]).astype(bf)
    stab = np.sin(pos[None, :] * invf2s[:, None]).astype(bf)
    qnw = (np.asarray(q_norm_w, np.float32) / math.sqrt(HD)).reshape(128, 1)
    knw = np.asarray(k_norm_w, np.float32).reshape(128, 1)

    ones = np.ones((128, 128), bf)
    tri01 = np.where(np.arange(128)[:, None] <= np.arange(128)[None, :],
                     1.0, 0.0).astype(bf)
    return dict(xt=xt, wq=wq, wk=wk, wv=wv, wo=wo, ctab=ctab, stab=stab,
                qnw=qnw, knw=knw, ones=ones, tri01=tri01)


def kernel(hidden_states, position_ids, q_w, k_w, v_w, o_w, q_norm_w,
           k_norm_w):
    from concourse.bass_utils import run_bass_kernel_spmd

    nc = build_nc()
    orig_ser = nc.to_json_bytes
    nc.to_json_bytes = lambda: legalize_waits(orig_ser())
    in_maps = [
        _prep_core_inputs(c, hidden_states, position_ids, q_w, k_w, v_w, o_w,
                          q_norm_w, k_norm_w)
        for c in range(NCORES)
    ]
    res = run_bass_kernel_spmd(nc, in_maps, list(range(NCORES))).results
    out = np.zeros((B, L, D), np.float32)
    for c in range(NCORES):
        out[c // TPG] += np.asarray(res[c]["out"], np.float32).reshape(L, D)
    return out


# revision 12
# speedup vs baseline: 237.9162x; 1.1182x over previous
"""Bass/Trainium2 kernel for nn_Attention_19481971654841.

Full attention block: q/k/v proj + per-head RMSNorm(q,k) + RoPE + causal GQA
SDPA + o_proj.  B=2, L=2048, D=1024, H=16, KVH=8, HD=128.

Sharding (8 NeuronCores): data-parallel over batch (2 groups of 4 cores) x
4-way tensor-parallel over heads inside each group.  Core c handles batch
c//4 and q-heads [4g:4g+4), kv-heads [2g:2g+2) with g = c%4.  Each core
produces a partial [L, D] o_proj contribution; host sums the 4 partials per
batch.

Per-core dataflow (all matmuls bf16, fp32 PSUM accumulate), in three strict
phases chosen so the ACT engine never thrashes activation tables (Sqrt and
Exp live in different tables; a switch costs 1.3us):

Phase 1 - projections + norm + rope (PE-bound, ~60us):
  - Q^T/K^T head-major [HD=128 part, T] (weights stationary); V token-major
    via X-stationary matmuls.
  - RMSNorm: sum-of-squares via all-ones matmul (partition reduce broadcast
    to 128 rows), ACT Sqrt (batched - one table load), DVE reciprocal,
    fused (q * w) * r scalar_tensor_tensor.
  - RoPE: host-prepared bf16 cos/sin tables (sin carries the rotate-half
    sign); rotate-half itself is a partition swap done by SBUF->SBUF DMA,
    so rope is 2 fast DVE muls + a Pool add per plane unit.

Phase 2 - attention (PE/ACT-balanced, ~100us), S^T orientation:
  scores[kv, q] = K_hm^T . Q_hm -> ACT Exp (one table load, accum-free) ->
  P^T bf16; causal diagonal masked by a Pool-engine multiply with an
  upper-triangular 0/1 block AFTER exp (no PE mask matmuls); softmax
  denominator via all-ones matmul; PV accumulates attn^T[hd, q] directly.
  All 96 (head, qtile, kvblock) units flow through one software pipeline
  (scores of pair i+1 emitted before denom/PV of pair i) with no per-head
  drains.

Phase 3 - o_proj tail (~28us): attn^T slices stationary, fp32 partial out
  DMA'd straight from PSUM to DRAM (no drain copies), double-buffered.
"""

import math
from contextlib import ExitStack

import numpy as np
import ml_dtypes

import concourse.bass as bass
import concourse.mybir as mybir
import concourse.tile as tile

F32 = mybir.dt.float32
BF16 = mybir.dt.bfloat16
AF = mybir.ActivationFunctionType
ALU = mybir.AluOpType

# problem constants (per spec; hardcoded — kernel.py must be self-contained)
B, L, D = 2, 2048, 1024
H, KVH, HD = 16, 8, 128
EPS = 1e-6
ROPE_BASE = 1000000.0

# per-core constants
NCORES = 8
TPG = 4                 # tensor-parallel group size (cores per batch)
T = L                   # tokens per core (one batch element)
NQ = H // TPG           # 4 q heads per core
NKV = KVH // TPG        # 2 kv heads per core
DCH = D // 128          # 8 input-dim chunks
QT = 1024               # q tile width (PSUM tile [128, 1024] f32 = 2 banks)
NQT = T // QT           # 2 q tiles
NCH = T // 128          # 16 token chunks


def _chunks512(c0, end):
    """bank-aligned matmul col chunks covering [c0, end)"""
    out = []
    n0 = c0
    while n0 < end:
        ne = min(end, (n0 // 512 + 1) * 512)
        out.append((n0, ne))
        n0 = ne
    return out


def build_nc(nrep=1):
    """nrep > 1 replicates the whole kernel body (same SBUF buffers, fresh
    DRAM loads) for benchmarking: per-rep steady-state time = true HW exec
    time with dispatch overhead amortized away.  kernel() always uses
    nrep=1."""
    nc = bass.Bass()

    xt_d = nc.dram_tensor("xt", [DCH, 128, T], BF16, kind="ExternalInput")
    wq_d = nc.dram_tensor("wq", [DCH, 128, NQ * HD], BF16, kind="ExternalInput")
    wk_d = nc.dram_tensor("wk", [DCH, 128, NKV * HD], BF16, kind="ExternalInput")
    wv_d = nc.dram_tensor("wv", [DCH, 128, NKV * HD], BF16, kind="ExternalInput")
    wo_d = nc.dram_tensor("wo", [NQ, 128, D], BF16, kind="ExternalInput")
    ctab_d = nc.dram_tensor("ctab", [128, T], BF16, kind="ExternalInput")
    stab_d = nc.dram_tensor("stab", [128, T], BF16, kind="ExternalInput")
    qnw_d = nc.dram_tensor("qnw", [128, 1], F32, kind="ExternalInput")
    knw_d = nc.dram_tensor("knw", [128, 1], F32, kind="ExternalInput")
    ones_d = nc.dram_tensor("ones", [128, 128], BF16, kind="ExternalInput")
    tri01_d = nc.dram_tensor("tri01", [128, 128], BF16, kind="ExternalInput")
    out_d = nc.dram_tensor("out", [NCH, 128, D], F32, kind="ExternalOutput")

    with tile.TileContext(nc) as tc, ExitStack() as ctx:
        sing = ctx.enter_context(tc.tile_pool(name="sing", bufs=1))
        trans = ctx.enter_context(tc.tile_pool(name="trans", bufs=2))
        pts = ctx.enter_context(tc.tile_pool(name="pts", bufs=4))
        psum = ctx.enter_context(tc.tile_pool(name="psum", bufs=1, space="PSUM"))

        for _rep in range(nrep):
            # ---- persistent loads (wk + xt chunk 0 first so PE starts early)
            wk = sing.tile([128, DCH, NKV * HD], BF16, tag="wk")
            nc.sync.dma_start(out=wk, in_=wk_d.rearrange("d p f -> p d f"))
            # xt split across the two HWDGE queues (SP + ACT) so the first
            # projection fronts aren't DMA-bandwidth starved
            xt = sing.tile([128, DCH, T], BF16, tag="xt")
            for d in range(DCH):
                eng = nc.sync if d % 2 == 0 else nc.scalar
                eng.dma_start(out=xt[:, d, :], in_=xt_d[d])
            wq = sing.tile([128, DCH, NQ * HD], BF16, tag="wq")
            wv = sing.tile([128, DCH, NKV * HD], BF16, tag="wv")
            nc.scalar.dma_start(out=wq, in_=wq_d.rearrange("d p f -> p d f"))
            nc.sync.dma_start(out=wv, in_=wv_d.rearrange("d p f -> p d f"))
            wo = sing.tile([128, NQ, D], BF16, tag="wo")
            nc.sync.dma_start(out=wo, in_=wo_d.rearrange("h p f -> p h f"))
            ctab = sing.tile([128, T], BF16, tag="ctab")
            stab = sing.tile([128, T], BF16, tag="stab")
            nc.sync.dma_start(out=ctab, in_=ctab_d[:, :])
            nc.sync.dma_start(out=stab, in_=stab_d[:, :])
            qnw = sing.tile([128, 1], F32, tag="qnw")
            knw = sing.tile([128, 1], F32, tag="knw")
            nc.sync.dma_start(out=qnw, in_=qnw_d[:, :])
            nc.sync.dma_start(out=knw, in_=knw_d[:, :])
            ones = sing.tile([128, 128], BF16, tag="ones")
            tri01 = sing.tile([128, 128], BF16, tag="tri01")
            nc.sync.dma_start(out=ones, in_=ones_d[:, :])
            nc.sync.dma_start(out=tri01, in_=tri01_d[:, :])
            epsb = sing.tile([128, 1], F32, tag="epsb")
            nc.vector.memset(epsb, EPS)

            # ---- persistent plane outputs -------------------------------
            khm = [sing.tile([128, T], BF16, tag=f"khm{i}", name=f"khm{i}")
                   for i in range(NKV)]
            qhm = [sing.tile([128, T], BF16, tag=f"qhm{i}", name=f"qhm{i}")
                   for i in range(NQ)]
            vsb = sing.tile([128, NKV, T], BF16, tag="vsb")
            attn = [sing.tile([128, T], BF16, tag=f"attn{i}", name=f"attn{i}")
                    for i in range(NQ)]

            # ---- phase 1: projections + norm + rope ---------------------
            # Unit = (plane, half).  Front: projection matmuls + PSUM drain
            # to bf16 + square.  Back: sum-of-squares ones-matmul, ACT Sqrt,
            # DVE reciprocal + fused scale, rope (DMA partition swap + DVE
            # muls + Pool add).
            def unit_front(w_ap, wmat, fslice, half):
                qp = psum.tile([128, QT], F32, tag="pp", bufs=2, name="qp")
                for d in range(DCH):
                    for n0 in range(0, QT, 512):
                        nc.tensor.matmul(
                            qp[:, n0:n0 + 512],
                            lhsT=wmat[:, d, fslice],
                            rhs=xt[:, d, half * QT + n0: half * QT + n0 + 512],
                            start=(d == 0), stop=(d == DCH - 1),
                        )
                # drain qp on ACT (Square + Copy live in the Sqrt table, so
                # phase 1 stays on one ACT table); DVE keeps the norm chain
                sq = trans.tile([128, QT], BF16, tag="sq", bufs=2, name="sq")
                nc.scalar.activation(out=sq, in_=qp, func=AF.Square)
                qc = trans.tile([128, QT], BF16, tag="qc", bufs=3, name="qc")
                nc.scalar.activation(out=qc, in_=qp, func=AF.Copy)
                return (w_ap, half, qc, sq)

            def unit_back(plane_out, st):
                w_ap, half, qc, sq = st
                ssq = psum.tile([128, QT], F32, tag="pd", bufs=1, name="ssq")
                for n0 in range(0, QT, 512):
                    nc.tensor.matmul(ssq[:, n0:n0 + 512], lhsT=ones,
                                     rhs=sq[:, n0:n0 + 512],
                                     start=True, stop=True)
                ss = trans.tile([128, QT], F32, tag="ss", name="ss")
                nc.scalar.activation(out=ss, in_=ssq, func=AF.Sqrt,
                                     scale=1.0 / HD, bias=epsb)
                rr = trans.tile([128, QT], F32, tag="rr", name="rr")
                nc.vector.reciprocal(out=rr, in_=ss)
                qn = trans.tile([128, QT], BF16, tag="qn", name="qn")
                nc.vector.scalar_tensor_tensor(
                    out=qn, in0=qc, scalar=w_ap, in1=rr,
                    op0=ALU.mult, op1=ALU.mult)
                cs = slice(half * QT, half * QT + QT)
                mc = trans.tile([128, QT], BF16, tag="mc", name="mc")
                nc.vector.tensor_mul(mc, qn, ctab[:, cs])
                # rotate-half: partition swap via SBUF->SBUF DMA; the sign
                # lives in the sin table (stab rows 0:64 hold -sin)
                qnsw = trans.tile([128, QT], BF16, tag="qnsw", name="qnsw")
                nc.sync.dma_start(out=qnsw[0:64, :], in_=qn[64:128, :])
                nc.sync.dma_start(out=qnsw[64:128, :], in_=qn[0:64, :])
                msw = trans.tile([128, QT], BF16, tag="msw", name="msw")
                nc.vector.tensor_mul(msw, qnsw, stab[:, cs])
                nc.vector.tensor_add(plane_out[:, cs], mc, msw)

            units = [(khm[i], knw, wk, slice(i * HD, (i + 1) * HD), half)
                     for i in range(NKV) for half in range(NQT)]
            units += [(qhm[i], qnw, wq, slice(i * HD, (i + 1) * HD), half)
                      for i in range(NQ) for half in range(NQT)]
            pend = []
            for plane, w_ap, wmat, fsl, half in units:
                st = unit_front(w_ap, wmat, fsl, half)
                pend.append((plane, st))
                if len(pend) > 1:
                    unit_back(*pend.pop(0))
            while pend:
                unit_back(*pend.pop(0))

            # V projection: token-major via X-stationary matmuls
            for c in range(NCH):
                vp = psum.tile([128, NKV * HD], F32, tag="pacc", bufs=1,
                               name="vp")
                for d in range(DCH):
                    nc.tensor.matmul(
                        vp, lhsT=xt[:, d, c * 128:(c + 1) * 128],
                        rhs=wv[:, d, :], start=(d == 0), stop=(d == DCH - 1))
                nc.scalar.activation(
                    out=vsb[:, :, c * 128:(c + 1) * 128],
                    in_=vp.rearrange("p (k t) -> p k t", k=NKV),
                    func=AF.Copy)

            # ---- phase 2: attention, one global software pipeline -------
            # Block = (h, iqt, j).  Pairs (j, j+1) within a (h, iqt) unit;
            # scores+exp of pair i+1 are emitted before denom/PV of pair i
            # so PE never waits on ACT.  ps_o/ps_d live per (h, iqt).
            st_ctx = {}

            def scores(h, iqt, j):
                kv = h // 2
                c0 = max(0, 128 * j - QT * iqt)
                c0a = (c0 // 512) * 512   # bank-aligned start for denom/PV
                ks = slice(128 * j, 128 * j + 128)
                ps_s = psum.tile([128, QT], F32, tag="pp", bufs=2, name="ps_s")
                for n0, ne in _chunks512(c0, QT):
                    nc.tensor.matmul(
                        ps_s[:, n0:ne], lhsT=khm[kv][:, ks],
                        rhs=qhm[h][:, iqt * QT + n0: iqt * QT + ne],
                        start=True, stop=True)
                pt = pts.tile([128, QT], BF16, tag="pt", name="pt")
                if c0 > c0a:
                    # denom/PV read bank-aligned [c0a:); zero the pad region
                    nc.vector.memset(pt[:, c0a:c0], 0.0)
                nc.scalar.activation(out=pt[:, c0:QT], in_=ps_s[:, c0:QT],
                                     func=AF.Exp)
                if j >= 8 * iqt:
                    # causal diagonal: zero the strictly-lower block entries
                    # (DVE, not Pool: gpsimd measures ~2.5x slower than its
                    # cost model on HW and this is on the pt critical path)
                    nc.vector.tensor_mul(pt[:, c0:c0 + 128],
                                         pt[:, c0:c0 + 128], tri01)
                return j, c0a, pt

            def denoms(h, iqt, st):
                j, c0a, pt = st
                jmax = 8 * iqt + 8
                if (h, iqt, "d") not in st_ctx:
                    # lazy alloc: first write lands after the previous
                    # unit's reciprocal (same pd buffer) was emitted
                    st_ctx[(h, iqt, "d")] = psum.tile(
                        [128, QT], F32, tag="pd", bufs=1, name="ps_d")
                ps_d = st_ctx[(h, iqt, "d")]
                for n0, ne in _chunks512(c0a, QT):
                    jl = min(jmax - 1, 8 * iqt + n0 // 128 + 3)
                    nc.tensor.matmul(ps_d[:, n0:ne], lhsT=ones,
                                     rhs=pt[:, n0:ne],
                                     start=(j == 0), stop=(j == jl))

            def pvs(h, iqt, st):
                j, c0a, pt = st
                jmax = 8 * iqt + 8
                kv = h // 2
                if (h, iqt, "o") not in st_ctx:
                    st_ctx[(h, iqt, "o")] = psum.tile(
                        [128, QT], F32, tag="pacc", bufs=1, name="ps_o")
                ps_o = st_ctx[(h, iqt, "o")]
                kvs = slice(128 * j, 128 * j + 128)
                for n0, ne in _chunks512(c0a, QT):
                    jl = min(jmax - 1, 8 * iqt + n0 // 128 + 3)
                    nc.tensor.matmul(ps_o[:, n0:ne],
                                     lhsT=vsb[:, kv, kvs],
                                     rhs=pt[:, n0:ne],
                                     start=(j == 0), stop=(j == jl))

            def finish_unit(h, iqt):
                ps_d = st_ctx.pop((h, iqt, "d"))
                ps_o = st_ctx.pop((h, iqt, "o"))
                rb = trans.tile([128, QT], F32, tag="rb", name="rb")
                nc.vector.reciprocal(out=rb, in_=ps_d)
                nc.vector.tensor_mul(
                    attn[h][:, iqt * QT:(iqt + 1) * QT], ps_o, rb)

            pairs = []
            for h in range(NQ):
                for iqt in range(NQT):
                    js = list(range(8 * iqt + 8))
                    pairs += [(h, iqt, js[i], js[i + 1])
                              for i in range(0, len(js), 2)]

            prev = None
            for h, iqt, j0, j1 in pairs:
                s0 = scores(h, iqt, j0)
                s1 = scores(h, iqt, j1)
                if prev is not None:
                    ph, piqt, p0, p1 = prev
                    denoms(ph, piqt, p0)
                    denoms(ph, piqt, p1)
                    pvs(ph, piqt, p0)
                    pvs(ph, piqt, p1)
                    if p1[0] == 8 * piqt + 8 - 1:  # last pair of unit
                        finish_unit(ph, piqt)
                prev = (h, iqt, s0, s1)
            ph, piqt, p0, p1 = prev
            denoms(ph, piqt, p0)
            denoms(ph, piqt, p1)
            pvs(ph, piqt, p0)
            pvs(ph, piqt, p1)
            finish_unit(ph, piqt)

            # ---- phase 3: o_proj; ACT (idle here, Copy is in the exp
            # table) drains PSUM, DMA ships to DRAM, double-buffered ------
            for c in range(NCH):
                po = psum.tile([128, D], F32, tag="pp", bufs=2, name="po")
                ts = slice(c * 128, (c + 1) * 128)
                for hh in range(NQ):
                    for n0 in range(0, D, 512):
                        nc.tensor.matmul(po[:, n0:n0 + 512],
                                         lhsT=attn[hh][:, ts],
                                         rhs=wo[:, hh, n0:n0 + 512],
                                         start=(hh == 0), stop=(hh == NQ - 1))
                ob = pts.tile([128, D], F32, tag="ob", bufs=2, name="ob")
                nc.scalar.activation(out=ob, in_=po, func=AF.Copy)
                nc.sync.dma_start(out=out_d[c], in_=ob)

    return nc


def legalize_waits(bir_bytes):
    """This walrus build rejects compute instructions with more than one
    sync wait.  Hoist all but one wait of each instruction into standalone
    EventSemaphore (pure wait) instructions on the same engine queue, which
    is semantically identical (in-order engine queues)."""
    import json
    m = json.loads(bir_bytes)
    n_fix = 0
    for f in m["functions"]:
        for blk in f["blocks"]:
            # drop Ldweights identical to the previously-kept one (the
            # stationary operand is still loaded; bass re-emits per matmul).
            # Safe: Ldweights carry no on_update; waits (rare) are kept.
            out0 = []
            last_key = None
            for ins in blk["instructions"]:
                if ins["opcode"] == "Ldweights":
                    si = ins.get("sync_info") or {}
                    key = json.dumps(
                        [ins.get("ins"), ins.get("outs"),
                         ins.get("perf_mode"), ins.get("tile_position")])
                    if (key == last_key and not si.get("on_wait")
                            and not si.get("on_update")):
                        continue
                    last_key = key
                out0.append(ins)
            blk["instructions"] = out0
            out = []
            for ins in blk["instructions"]:
                si = ins.get("sync_info")
                waits = (si or {}).get("on_wait") or []
                if len(waits) > 1 and ins["opcode"] != "EventSemaphore":
                    for i, w in enumerate(waits[:-1]):
                        out.append({
                            "debug": ins.get("debug", 0),
                            "engine": ins["engine"],
                            "ins": [], "outs": [],
                            "name": f"{ins['name']}-hw{i}",
                            "opcode": "EventSemaphore",
                            "sync_info": {"on_update": [], "on_wait": [w]},
                        })
                    si["on_wait"] = [waits[-1]]
                    n_fix += 1
                out.append(ins)
            blk["instructions"] = out
    return json.dumps(m).encode()


def _prep_core_inputs(c, hidden_states, position_ids, q_w, k_w, v_w, o_w,
                      q_norm_w, k_norm_w):
    b, g = divmod(c, TPG)
    bf = ml_dtypes.bfloat16
    xt = np.ascontiguousarray(
        np.asarray(hidden_states[b], np.float32).T).astype(bf).reshape(DCH, 128, T)
    wq = np.ascontiguousarray(
        np.asarray(q_w[NQ * HD * g: NQ * HD * (g + 1)], np.float32).T
    ).astype(bf).reshape(DCH, 128, NQ * HD)
    wk = np.ascontiguousarray(
        np.asarray(k_w[NKV * HD * g: NKV * HD * (g + 1)], np.float32).T
    ).astype(bf).reshape(DCH, 128, NKV * HD)
    wv = np.ascontiguousarray(
        np.asarray(v_w[NKV * HD * g: NKV * HD * (g + 1)], np.float32).T
    ).astype(bf).reshape(DCH, 128, NKV * HD)
    wo = np.ascontiguousarray(
        np.asarray(o_w[:, NQ * HD * g: NQ * HD * (g + 1)], np.float32).T
    ).astype(bf).reshape(NQ, 128, D)

    pos = np.asarray(position_ids[b], np.float64)                      # [T]
    inv = 1.0 / (ROPE_BASE ** (np.arange(0, HD, 2, dtype=np.float64) / HD))
    invf2 = np.concatenate([inv, inv])                                 # [128]
    invf2s = np.concatenate([-inv, inv])
    ctab = np.cos(pos[None, :] * invf2[:, None]).astype(bf)
    stab = np.sin(pos[None, :] * invf2s[:, None]).astype(bf)
    qnw = (np.asarray(q_norm_w, np.float32) / math.sqrt(HD)).reshape(128, 1)
    knw = np.asarray(k_norm_w, np.float32).reshape(128, 1)

    ones = np.ones((128, 128), bf)
    tri01 = np.where(np.arange(128)[:, None] <= np.arange(128)[None, :],
                     1.0, 0.0).astype(bf)
    return dict(xt=xt, wq=wq, wk=wk, wv=wv, wo=wo, ctab=ctab, stab=stab,
                qnw=qnw, knw=knw, ones=ones, tri01=tri01)


def kernel(hidden_states, position_ids, q_w, k_w, v_w, o_w, q_norm_w,
           k_norm_w):
    from concourse.bass_utils import run_bass_kernel_spmd

    nc = build_nc()
    orig_ser = nc.to_json_bytes
    nc.to_json_bytes = lambda: legalize_waits(orig_ser())
    in_maps = [
        _prep_core_inputs(c, hidden_states, position_ids, q_w, k_w, v_w, o_w,
                          q_norm_w, k_norm_w)
        for c in range(NCORES)
    ]
    res = run_bass_kernel_spmd(nc, in_maps, list(range(NCORES))).results
    out = np.zeros((B, L, D), np.float32)
    for c in range(NCORES):
        out[c // TPG] += np.asarray(res[c]["out"], np.float32).reshape(L, D)
    return out
